# revision 1
# baseline (speedup 1.0000x reference)
"""Multi-head causal attention with RoPE on 8 Trainium2 NeuronCores.

Sharding: 2 (batch) x 4 (head-groups of 4 heads). Each core computes
QKV projections, RoPE, flash-style causal attention and its slice of the
output projection for one batch and 4 heads; partial outputs are summed
on the host (row-sharded out_proj => partial-sum reduction).

Device layout choices (everything host-prepped to avoid on-device
transposes, fp32 has no DMA-transpose path):
  - x is passed pre-transposed per batch: xT [D, S] bf16
  - Q^T, K^T computed as [head_dim, S] (lhsT = W tile, rhs = xT)
  - V computed natural [S, head_dim] (lhsT = xT tile, rhs = Wv)
  - scores computed transposed [k, q]; softmax sum over k (partitions)
    via a full-width all-ones stationary matmul, which lands the same sum
    on every PSUM partition so normalization needs no broadcast
  - RoPE rotate-half done with a signed 128x128 permutation matmul
  - attention g-outer with per-q-group out-proj interleaved; scores
    pipelined one k-tile ahead of the PV/sum matmuls
"""

import math
import sys

import numpy as np

try:
    import concourse.bass as bass  # noqa: F401
except Exception:
    sys.path.insert(0, "/opt/trn_rl_repo")

import ml_dtypes

P = 128
B = 2
S = 2048
D = 2048
H = 16
HEAD = 128
N_CORES = 8
HG = 4            # head groups (tensor-parallel dimension)
HPG = H // HG     # heads per group = 4
DG = HPG * HEAD   # group width = 512
SG = 512          # q-group (free dim) size
DOUT = 2048

BF16 = ml_dtypes.bfloat16


def _emit(tc, io, cfg, sfx=""):
    """Emit the per-core program. io: dict of dram APs. cfg: sizes."""
    import concourse.mybir as mybir

    nc = tc.nc
    bf = mybir.dt.bfloat16
    f32 = mybir.dt.float32
    Exp = mybir.ActivationFunctionType.Exp

    s = cfg["S"]
    d = cfg["D"]
    dout = cfg["DOUT"]
    di_t = d // P          # d_in k-tiles
    st = s // P            # seq 128-tiles
    nsg = s // SG          # seq 512-groups
    nos = dout // SG       # out column slices
    inv_sqrt_hd = 1.0 / math.sqrt(HEAD)

    xT = io["xT"].rearrange("(o p) s -> p o s", p=P)
    wq = io["wq"].rearrange("(o p) n -> p o n", p=P)
    wk = io["wk"].rearrange("(o p) n -> p o n", p=P)
    wv = io["wv"].rearrange("(o p) n -> p o n", p=P)
    wo = io["wo"].rearrange("(o p) n -> p o n", p=P)

    const = tc.alloc_tile_pool(name="const" + sfx, bufs=1)
    stores = tc.alloc_tile_pool(name="stores" + sfx, bufs=1)
    ps_main = tc.alloc_tile_pool(name="ps_main" + sfx, bufs=3, space="PSUM")
    ps2 = tc.alloc_tile_pool(name="ps2" + sfx, bufs=2, space="PSUM")
    ps_sum = tc.alloc_tile_pool(name="ps_sum" + sfx, bufs=1, space="PSUM")

    # ---- constants (tiles only; DMAs emitted after the xT stream) ----
    cos_sb = const.tile([P, s], bf, tag="cos")
    sin_sb = const.tile([P, s], bf, tag="sin")
    rot_sb = const.tile([P, P], bf, tag="rot")
    mask_sb = const.tile([P, HG, SG], bf, tag="mask")
    ones_bf_sb = const.tile([P, P], bf, tag="ones_bf")
    wv_sb = const.tile([P, di_t, DG], bf, tag="wv")
    wo_sb = const.tile([P, HPG, dout], bf, tag="wo")

    # persistent activation stores
    qt_sb = stores.tile([P, HPG, s], bf, tag="qt")
    kt_sb = stores.tile([P, HPG, s], bf, tag="kt")
    v_sb = stores.tile([P, st, DG], bf, tag="v")
    ctx_sb = stores.tile([P, HPG, s], bf, tag="ctx")

    # ---- phase 1: projections + RoPE ----
    with tc.tile_pool(name="xt" + sfx, bufs=1) as xtp, \
         tc.tile_pool(name="wqk" + sfx, bufs=2) as wqkp, \
         tc.tile_pool(name="p1tmp" + sfx, bufs=4) as p1tmp:
        xt_sb = xtp.tile([P, di_t, s], bf, tag="xt")
        # wv first (V needs it), then xT by column-group so early V/QK tiles
        # land fast; remaining constants follow.
        for o in range(di_t):
            nc.sync.dma_start(wv_sb[:, o, :], wv[:, o, :])
            nc.sync.dma_start(xt_sb[:, o, 0:SG], xT[:, o, 0:SG])
        for g in range(1, nsg):
            for o in range(di_t):
                nc.sync.dma_start(
                    xt_sb[:, o, g * SG:(g + 1) * SG], xT[:, o, g * SG:(g + 1) * SG]
                )
        nc.sync.dma_start(cos_sb[:], io["cosT"][:])
        nc.sync.dma_start(sin_sb[:], io["sinT"][:])
        nc.sync.dma_start(rot_sb[:], io["rot"][:])
        nc.sync.dma_start(mask_sb[:], io["masks"][:])
        nc.sync.dma_start(ones_bf_sb[:], io["ones_bf"][:])
        for o in range(HPG):
            nc.sync.dma_start(wo_sb[:, o, :], wo[:, o, :])

        # V natural layout: [s_tile, DG]
        for si in range(st):
            pv = ps_main.tile([P, SG], f32, tag="ps")
            for o in range(di_t):
                nc.tensor.matmul(
                    pv[:, :DG],
                    lhsT=xt_sb[:, o, si * P:(si + 1) * P],
                    rhs=wv_sb[:, o, :],
                    start=(o == 0),
                    stop=(o == di_t - 1),
                )
            nc.vector.tensor_copy(v_sb[:, si, :], pv[:, :DG])

        # Q^T, K^T with RoPE, per head; rot-MM pipelined behind the next
        # projection block so the PE never waits on the ACT psum->sbuf copy
        for h in range(HPG):
            wq_t = wqkp.tile([P, di_t, P], bf, tag="wq")
            wk_t = wqkp.tile([P, di_t, P], bf, tag="wk")
            for o in range(di_t):
                nc.sync.dma_start(wq_t[:, o, :], wq[:, o, h * P:(h + 1) * P])
                nc.sync.dma_start(wk_t[:, o, :], wk[:, o, h * P:(h + 1) * P])

            def emit_rope(qa, dst, hh, sl):
                pr = ps_main.tile([P, SG], f32, tag="ps")
                nc.tensor.matmul(pr, lhsT=rot_sb, rhs=qa, start=True, stop=True)
                t1 = p1tmp.tile([P, SG], bf, tag="t1")
                nc.vector.tensor_mul(t1, qa, cos_sb[:, sl])
                t2 = p1tmp.tile([P, SG], bf, tag="t2")
                nc.vector.tensor_mul(t2, pr, sin_sb[:, sl])
                nc.vector.tensor_add(dst[:, hh, sl], t1, t2)

            pending = []
            for g in range(nsg):
                sl = slice(g * SG, (g + 1) * SG)
                for w_t, dst in ((wq_t, qt_sb), (wk_t, kt_sb)):
                    pq = ps_main.tile([P, SG], f32, tag="ps")
                    for o in range(di_t):
                        nc.tensor.matmul(
                            pq,
                            lhsT=w_t[:, o, :],
                            rhs=xt_sb[:, o, sl],
                            start=(o == 0),
                            stop=(o == di_t - 1),
                        )
                    qa = p1tmp.tile([P, SG], bf, tag="qa")
                    nc.scalar.copy(qa, pq)
                    pending.append((qa, dst, h, sl))
                    while len(pending) > 2:
                        emit_rope(*pending.pop(0))
            while pending:
                emit_rope(*pending.pop(0))

    # ---- phase 2+3: attention interleaved with output projection ----
    # g outer so each q-group's out-proj tiles become ready early and fill
    # the PE while later q-groups' softmax runs. Scores pipelined one k-tile
    # ahead of PV; softmax sum uses a full-width ones stationary so the
    # normalization needs no cross-partition broadcast.
    with tc.tile_pool(name="p2tmp" + sfx, bufs=10) as p2tmp, \
         tc.tile_pool(name="p2rb" + sfx, bufs=3) as p2rb, \
         tc.tile_pool(name="outp" + sfx, bufs=3) as outp:
        for g in range(nsg):
            qsl = slice(g * SG, (g + 1) * SG)
            jmax = min((g + 1) * SG // P, st)
            for h in range(HPG):
                pctx = ps_main.tile([P, SG], f32, tag="ps")
                psum_l = ps_sum.tile([P, SG], f32, tag="l")

                # stream 1: paired score MMs + one exp per [P, 2*SG];
                # DVE pre-reduces each pair so the softmax-sum matmul
                # stream is halved
                ats = []
                dsums = []
                for j in range(0, jmax, 2):
                    ps2t = ps2.tile([P, 2, SG], f32, tag="ps2")
                    nc.tensor.matmul(
                        ps2t[:, 0, :],
                        lhsT=kt_sb[:, h, j * P:(j + 1) * P],
                        rhs=qt_sb[:, h, qsl],
                        start=True,
                        stop=True,
                    )
                    nc.tensor.matmul(
                        ps2t[:, 1, :],
                        lhsT=kt_sb[:, h, (j + 1) * P:(j + 2) * P],
                        rhs=qt_sb[:, h, qsl],
                        start=True,
                        stop=True,
                    )
                    at2 = p2tmp.tile([P, 2, SG], bf, tag="at")
                    nc.scalar.activation(at2, ps2t, Exp, scale=inv_sqrt_hd)
                    r = j - (g * SG // P)
                    if r >= 0:
                        nc.vector.tensor_mul(at2, at2, mask_sb[:, r:r + 2, :])
                    ats.append(at2)
                    dsum = p2tmp.tile([P, SG], bf, tag="dsum")
                    nc.vector.tensor_add(dsum, at2[:, 0, :], at2[:, 1, :])
                    dsums.append(dsum)

                # stream 2: PV accumulation (wait-free after exps drain)
                for idx, at2 in enumerate(ats):
                    for jj in range(2):
                        j = 2 * idx + jj
                        nc.tensor.matmul(
                            pctx,
                            lhsT=v_sb[:, j, h * P:(h + 1) * P],
                            rhs=at2[:, jj, :],
                            start=(j == 0),
                            stop=(j == jmax - 1),
                        )
                # second DVE tree level: combine pair-sums, then one
                # softmax-sum matmul per four k-tiles
                dsums2 = []
                for i in range(0, len(dsums), 2):
                    if i + 1 < len(dsums):
                        d2 = p2tmp.tile([P, SG], bf, tag="dsum2")
                        nc.vector.tensor_add(d2, dsums[i], dsums[i + 1])
                        dsums2.append(d2)
                    else:
                        dsums2.append(dsums[i])
                for idx, dsum in enumerate(dsums2):
                    nc.tensor.matmul(
                        psum_l,
                        lhsT=ones_bf_sb[:],
                        rhs=dsum,
                        start=(idx == 0),
                        stop=(idx == len(dsums2) - 1),
                    )
                rec = p2rb.tile([P, SG], f32, tag="rec")
                nc.vector.reciprocal_approx_fast(rec, psum_l)
                nc.vector.tensor_mul(ctx_sb[:, h, qsl], pctx, rec)

            for qt in range(4 * g, 4 * (g + 1)):
                for dsl in range(nos):
                    po = ps_main.tile([P, SG], f32, tag="ps")
                    for h in range(HPG):
                        nc.tensor.matmul(
                            po,
                            lhsT=ctx_sb[:, h, qt * P:(qt + 1) * P],
                            rhs=wo_sb[:, h, dsl * SG:(dsl + 1) * SG],
                            start=(h == 0),
                            stop=(h == HPG - 1),
                        )
                    ob = outp.tile([P, SG], f32, tag="ob")
                    nc.vector.tensor_copy(ob, po)
                    nc.sync.dma_start(
                        io["out"][qt * P:(qt + 1) * P, dsl * SG:(dsl + 1) * SG], ob
                    )

    for pool in (ps_sum, ps2, ps_main, stores, const):
        pool.release()


def build_program(cfg=None):
    import concourse.bacc as bacc
    import concourse.mybir as mybir
    import concourse.tile as tile

    cfg = cfg or {"S": S, "D": D, "DOUT": DOUT}
    bf = mybir.dt.bfloat16
    f32 = mybir.dt.float32
    nc = bacc.Bacc()
    io = {
        "xT": nc.dram_tensor("xT", [cfg["D"], cfg["S"]], bf, kind="ExternalInput"),
        "wq": nc.dram_tensor("wq", [cfg["D"], DG], bf, kind="ExternalInput"),
        "wk": nc.dram_tensor("wk", [cfg["D"], DG], bf, kind="ExternalInput"),
        "wv": nc.dram_tensor("wv", [cfg["D"], DG], bf, kind="ExternalInput"),
        "wo": nc.dram_tensor("wo", [DG, cfg["DOUT"]], bf, kind="ExternalInput"),
        "cosT": nc.dram_tensor("cosT", [P, cfg["S"]], bf, kind="ExternalInput"),
        "sinT": nc.dram_tensor("sinT", [P, cfg["S"]], bf, kind="ExternalInput"),
        "rot": nc.dram_tensor("rot", [P, P], bf, kind="ExternalInput"),
        "masks": nc.dram_tensor("masks", [P, HG, SG], bf, kind="ExternalInput"),
        "ones_bf": nc.dram_tensor("ones_bf", [P, P], bf, kind="ExternalInput"),
        "ones_f": nc.dram_tensor("ones_f", [1, P], f32, kind="ExternalInput"),
        "out": nc.dram_tensor(
            "out", [cfg["S"], cfg["DOUT"]], f32, kind="ExternalOutput"
        ),
    }
    with tile.TileContext(nc) as tc:
        for rep in range(cfg.get("repeat", 1)):
            _emit(tc, io, cfg, sfx=f"_r{rep}")
    nc.finalize()
    return nc


def host_constants(s=S):
    inv = 1.0 / (10000.0 ** (np.arange(0, HEAD, 2, dtype=np.float32) / HEAD))
    pos = np.arange(s, dtype=np.float32)
    ang = pos[:, None] * inv[None, :]
    ang = np.concatenate([ang, ang], axis=-1)          # (s, HEAD)
    cosT = np.cos(ang).T.astype(BF16).copy()           # (HEAD, s)
    sinT = np.sin(ang).T.astype(BF16).copy()
    rot = np.zeros((HEAD, HEAD), np.float32)
    for dd in range(64):
        rot[dd, dd + 64] = -1.0
        rot[dd + 64, dd] = 1.0
    rotT = rot.T.astype(BF16).copy()                   # lhsT for out = rot @ q
    kk = np.arange(P)[:, None, None]
    rr = np.arange(HG)[None, :, None]
    qq = np.arange(SG)[None, None, :]
    masks = (kk <= qq - P * rr).astype(BF16)           # (P, HG, SG)
    ones_bf = np.ones((P, P), BF16)
    ones_f = np.ones((1, P), np.float32)
    return cosT, sinT, rotT, masks, ones_bf, ones_f


def kernel(x, W_query, W_key, W_value, W_out):
    from concourse.bass_utils import run_bass_kernel_spmd

    x = np.asarray(x)
    in_dtype = x.dtype
    nc = build_program()
    cosT, sinT, rotT, masks, ones_bf, ones_f = host_constants()

    xTb = [np.ascontiguousarray(np.asarray(x[b]).T).astype(BF16) for b in range(B)]
    in_maps = []
    for core in range(N_CORES):
        b, g = divmod(core, HG)
        gsl = slice(g * DG, (g + 1) * DG)
        in_maps.append({
            "xT": xTb[b],
            "wq": np.asarray(W_query)[:, gsl].astype(BF16).copy(),
            "wk": np.asarray(W_key)[:, gsl].astype(BF16).copy(),
            "wv": np.asarray(W_value)[:, gsl].astype(BF16).copy(),
            "wo": np.asarray(W_out)[gsl, :].astype(BF16).copy(),
            "cosT": cosT, "sinT": sinT, "rot": rotT, "masks": masks,
            "ones_bf": ones_bf, "ones_f": ones_f,
        })

    res = run_bass_kernel_spmd(nc, in_maps, core_ids=list(range(N_CORES)))
    out = np.zeros((B, S, DOUT), np.float32)
    for core in range(N_CORES):
        b = core // HG
        out[b] += res.results[core]["out"]
    return out.astype(in_dtype, copy=False)



# revision 2
# speedup vs baseline: 1.0021x; 1.0021x over previous
"""Multi-head causal attention with RoPE on 8 Trainium2 NeuronCores — v2.

Sharding: 2 (batch) x 4 (head-groups of 4 heads), as v1. Differences vs v1,
all aimed at a stall-free PE stream (TRN2 HAM drops the PE clock to 1.2GHz
for ~3us after every stall):

  - o-major startup: QK projections for the first q-group run as 8 parallel
    PSUM accumulation chains fed by paired-o DMA, so the PE starts ~1.5us in
  - causal mask folded into the scores matmul as an additive -512 pattern
    (identity stationary), removing the DVE mask-multiply from the
    exp -> PV critical path
  - out-proj of group g-1 (and leftover V chains for g=0) interleaved into
    group g's exp-throttled scores stream as PE filler
  - bf16 output (host accumulates partial sums in f32)
  - batched DMA issues (sync sequencer pays ~617ns per dma_start)
"""

import math
import sys

import numpy as np

try:
    import concourse.bass as bass  # noqa: F401
except Exception:
    sys.path.insert(0, "/opt/trn_rl_repo")

import ml_dtypes

P = 128
B = 2
S = 2048
D = 2048
H = 16
HEAD = 128
N_CORES = 8
HG = 4            # head groups (tensor-parallel dimension)
HPG = H // HG     # heads per group = 4
DG = HPG * HEAD   # group width = 512
SG = 512          # q-group (free dim) size
DOUT = 2048

BF16 = ml_dtypes.bfloat16

MASK_NEG = -512.0   # pre-scale additive mask; exp(-512/sqrt(128)) ~ 3e-20


def _emit(tc, io, cfg, sfx=""):
    """Emit the per-core program. io: dict of dram APs. cfg: sizes."""
    import concourse.mybir as mybir

    nc = tc.nc
    bf = mybir.dt.bfloat16
    f16 = mybir.dt.float16
    f32 = mybir.dt.float32
    Exp = mybir.ActivationFunctionType.Exp

    s = cfg["S"]
    d = cfg["D"]
    dout = cfg["DOUT"]
    di_t = d // P          # d_in k-tiles (16)
    st = s // P            # seq 128-tiles (16)
    nsg = s // SG          # seq 512-groups (4)
    nos = dout // SG       # out column slices (4)
    inv_sqrt_hd = 1.0 / math.sqrt(HEAD)

    xT = io["xT"].rearrange("(o p) s -> p o s", p=P)
    wq = io["wq"].rearrange("(o p) n -> p o n", p=P)
    wk = io["wk"].rearrange("(o p) n -> p o n", p=P)
    wv = io["wv"].rearrange("(o p) n -> p o n", p=P)
    wo = io["wo"].rearrange("(o p) n -> p o n", p=P)

    const = tc.alloc_tile_pool(name="const" + sfx, bufs=1)
    stores = tc.alloc_tile_pool(name="stores" + sfx, bufs=1)

    # ---- constants ----
    cos_sb = const.tile([P, s], bf, tag="cos")
    sin_sb = const.tile([P, s], bf, tag="sin")
    rot_sb = const.tile([P, P], bf, tag="rot")
    mneg_sb = const.tile([P, HG, SG], bf, tag="mneg")
    id_sb = const.tile([P, P], bf, tag="id")
    ones_bf_sb = const.tile([P, P], bf, tag="ones_bf")
    wv_sb = const.tile([P, di_t, DG], bf, tag="wv")
    wo_sb = const.tile([P, HPG, dout], bf, tag="wo")

    # persistent activation stores. v is split per k-group so PV(g) reads
    # never false-share with later V-chain writes (subtile dep imprecision)
    qt_sb = stores.tile([P, HPG, s], bf, tag="qt")
    kt_sb = stores.tile([P, HPG, s], bf, tag="kt")
    v_g = [stores.tile([P, 4, DG], bf, tag=f"v{g}", name=f"v{g}")
           for g in range(nsg)]

    # ======================= phase 1 =======================
    ph1 = tc.alloc_tile_pool(name="ph1" + sfx, bufs=1)
    wqk = tc.alloc_tile_pool(name="wqk" + sfx, bufs=1)
    p1tmp = tc.alloc_tile_pool(name="p1tmp" + sfx, bufs=4)
    chp = tc.alloc_tile_pool(name="chp" + sfx, bufs=8, space="PSUM")

    # x^T split per s-group: chain reads of group g must not wait the DMA
    # of groups > g landing into the same tile
    xt_g = [ph1.tile([P, di_t, SG], bf, tag=f"xt{g}", name=f"xt{g}")
            for g in range(nsg)]
    wq_sb = wqk.tile([P, di_t, DG], bf, tag="wq")
    wk_sb = wqk.tile([P, di_t, DG], bf, tag="wk")

    # --- startup: o-major QK for g=0, all 4 heads, 8 psum chains;
    # paired-o DMA so the PE starts ~1.5us in and never waits again ---
    ch = {}
    for h in range(HPG):
        for qk in range(2):
            ch[(h, qk)] = chp.tile([P, SG], f32, tag="ch", name=f"ch{h}{qk}")
    for o in range(0, di_t, 2):
        if o == 0:  # single-o first group: first matmul starts sooner
            for oo in (0, 1):
                nc.sync.dma_start(xt_g[0][:, oo, :], xT[:, oo, 0:SG])
                nc.sync.dma_start(wq_sb[:, oo, :], wq[:, oo, :])
                nc.sync.dma_start(wk_sb[:, oo, :], wk[:, oo, :])
        else:
            nc.sync.dma_start(xt_g[0][:, o:o + 2, :], xT[:, o:o + 2, 0:SG])
            nc.sync.dma_start(wq_sb[:, o:o + 2, :], wq[:, o:o + 2, :])
            nc.sync.dma_start(wk_sb[:, o:o + 2, :], wk[:, o:o + 2, :])
        for oo in (o, o + 1):
            for h in range(HPG):
                for qk, w_sb in ((0, wq_sb), (1, wk_sb)):
                    nc.tensor.matmul(
                        ch[(h, qk)],
                        lhsT=w_sb[:, oo, h * P:(h + 1) * P],
                        rhs=xt_g[0][:, oo, :],
                        start=(oo == 0),
                        stop=(oo == di_t - 1),
                    )

    # remaining DMA (big batched issues; transfers stream behind compute).
    # wv first: the V chains that cover the startup->steady qa drain need it
    # ~25us before anything else in this list.
    nc.sync.dma_start(wv_sb[:], wv[:])
    nc.sync.dma_start(cos_sb[:], io["cosT"][:])
    nc.sync.dma_start(sin_sb[:], io["sinT"][:])
    nc.sync.dma_start(rot_sb[:], io["rot"][:])
    nc.sync.dma_start(mneg_sb[:], io["maskneg"][:])
    nc.sync.dma_start(id_sb[:], io["id"][:])
    nc.sync.dma_start(ones_bf_sb[:], io["ones_bf"][:])
    nc.sync.dma_start(xt_g[1][:], xT[:, :, SG:2 * SG])
    nc.sync.dma_start(wo_sb[:], wo[:])
    for g in range(2, nsg):
        nc.sync.dma_start(xt_g[g][:], xT[:, :, g * SG:(g + 1) * SG])

    # --- RoPE for the startup chains + steady-state QK for g>=1 ---
    def emit_rope(qa, dst, hh, sl):
        pr = chp.tile([P, SG], f32, tag="ch")
        nc.tensor.matmul(pr, lhsT=rot_sb, rhs=qa, start=True, stop=True)
        t1 = p1tmp.tile([P, SG], bf, tag="t1")
        nc.vector.tensor_mul(t1, qa, cos_sb[:, sl])
        t2 = p1tmp.tile([P, SG], bf, tag="t2")
        nc.vector.tensor_mul(t2, pr, sin_sb[:, sl])
        nc.vector.tensor_add(dst[:, hh, sl], t1, t2)

    pending = []

    def queue_rope(pq, dst, hh, sl, window=4):
        qa = p1tmp.tile([P, SG], bf, tag="qa")
        nc.scalar.copy(qa, pq)
        pending.append((qa, dst, hh, sl))
        while len(pending) > window:
            emit_rope(*pending.pop(0))

    def emit_v(si, pool):
        pv = pool.tile([P, SG], f32, tag=pool is chp and "ch" or "po", name="pv")
        sg, sc = divmod(si, 4)
        for o in range(di_t):
            nc.tensor.matmul(
                pv,
                lhsT=xt_g[sg][:, o, sc * P:(sc + 1) * P],
                rhs=wv_sb[:, o, :],
                start=(o == 0),
                stop=(o == di_t - 1),
            )
        nc.scalar.copy(v_g[sg][:, sc, :], pv)

    sl0 = slice(0, SG)
    for h in range(HPG):
        queue_rope(ch[(h, 0)], qt_sb, h, sl0, window=99)
        queue_rope(ch[(h, 1)], kt_sb, h, sl0, window=99)
    # two V chains give the PE qa-independent work while the eight startup
    # qa copies drain serially on ACT (wv was the first big DMA issued)
    emit_v(0, chp)
    while len(pending) > 6:
        emit_rope(*pending.pop(0))
    emit_v(1, chp)
    while len(pending) > 4:
        emit_rope(*pending.pop(0))

    for g in range(1, nsg):
        sl = slice(g * SG, (g + 1) * SG)
        for h in range(HPG):
            for qk, (w_sb, dst) in enumerate(((wq_sb, qt_sb), (wk_sb, kt_sb))):
                pq = chp.tile([P, SG], f32, tag="ch")
                for o in range(di_t):
                    nc.tensor.matmul(
                        pq,
                        lhsT=w_sb[:, o, h * P:(h + 1) * P],
                        rhs=xt_g[g][:, o, :],
                        start=(o == 0),
                        stop=(o == di_t - 1),
                    )
                queue_rope(pq, dst, h, sl)
    while pending:
        emit_rope(*pending.pop(0))

    # --- V projection si 2..11 (0-1 ran at the transition; 12-15 = g0 filler)
    for si in range(2, 12):
        emit_v(si, chp)

    chp.release()
    p1tmp.release()
    wqk.release()

    # ======================= phase 2 =======================
    ctxp = tc.alloc_tile_pool(name="ctx" + sfx, bufs=1)
    p2tmp = tc.alloc_tile_pool(name="p2tmp" + sfx, bufs=1)
    outp = tc.alloc_tile_pool(name="outp" + sfx, bufs=4)
    recp = tc.alloc_tile_pool(name="recp" + sfx, bufs=2)
    ps_pair = tc.alloc_tile_pool(name="ps_pair" + sfx, bufs=2, space="PSUM")
    ps_ctx = tc.alloc_tile_pool(name="ps_ctx" + sfx, bufs=1, space="PSUM")
    ps_l = tc.alloc_tile_pool(name="ps_l" + sfx, bufs=1, space="PSUM")
    ps_po = tc.alloc_tile_pool(name="ps_po" + sfx, bufs=2, space="PSUM")

    # ctx split per q-group: out-proj filler for g-1 must not false-share
    # with the current group's ctx writes
    ctx_g = [ctxp.tile([P, HPG, SG], bf, tag=f"ctx{g}", name=f"ctx{g}")
             for g in range(nsg)]

    # ---- filler pump: units of PE work with no exp dependency ----
    filler = []
    reserve = [0]   # units held back for later groups

    def pump(n):
        for _ in range(min(n, len(filler) - reserve[0])):
            filler.pop(0)()

    nob = [0]

    def po_unit(qt, dsl):
        def emit():
            po = ps_po.tile([P, SG], f32, tag="po")
            qg, qc = divmod(qt, 4)
            for h in range(HPG):
                nc.tensor.matmul(
                    po,
                    lhsT=ctx_g[qg][:, h, qc * P:(qc + 1) * P],
                    rhs=wo_sb[:, h, dsl * SG:(dsl + 1) * SG],
                    start=(h == 0),
                    stop=(h == HPG - 1),
                )
            ob = outp.tile([P, SG], bf, tag="ob")
            # alternate the psum->sbuf copy between DVE and ACT
            nob[0] += 1
            if nob[0] % 2:
                nc.vector.tensor_copy(ob, po)
            else:
                nc.scalar.copy(ob, po)
            nc.sync.dma_start(
                io["out"][qt * P:(qt + 1) * P, dsl * SG:(dsl + 1) * SG], ob
            )
        return emit

    for si in range(12, 16):
        filler.append(lambda si=si: emit_v(si, ps_po))

    for g in range(nsg):
        qsl = slice(g * SG, (g + 1) * SG)
        jmax = min((g + 1) * SG // P, st)
        # hold back filler in g2 so the exp-bound g3 stream stays fed
        reserve[0] = 8 if g == 2 else 0
        for h in range(HPG):
            pctx = ps_ctx.tile([P, SG], f32, tag="pctx")
            psum_l = ps_l.tile([P, SG], f32, tag="l")

            # scores stream: paired MMs + one exp per [P, 2*SG]; filler
            # absorbs the exp cadence. Diagonal tiles split into three MMs
            # (mask starter over masked cols, score accum on the boundary
            # 128, fresh scores right of it) — constant 640 cyc/diag tile,
            # every psum address start=True'd exactly once.
            ats = []
            dsums = []
            for j in range(0, jmax, 2):
                ps2t = ps_pair.tile([P, 2, SG], f32, tag="pair")
                for jj in range(2):
                    r = (j + jj) - (g * SG // P)
                    kt_t = kt_sb[:, h, (j + jj) * P:(j + jj + 1) * P]
                    if r < 0:
                        nc.tensor.matmul(
                            ps2t[:, jj, :], lhsT=kt_t, rhs=qt_sb[:, h, qsl],
                            start=True, stop=True,
                        )
                        continue
                    w = P * (r + 1)
                    q0 = g * SG
                    nc.tensor.matmul(
                        ps2t[:, jj, 0:w],
                        lhsT=id_sb, rhs=mneg_sb[:, r, 0:w],
                        start=True, stop=False, skip_group_check=True,
                    )
                    nc.tensor.matmul(
                        ps2t[:, jj, P * r:w],
                        lhsT=kt_t, rhs=qt_sb[:, h, q0 + P * r:q0 + w],
                        start=False, stop=(w == SG), skip_group_check=True,
                    )
                    if w < SG:
                        nc.tensor.matmul(
                            ps2t[:, jj, w:SG],
                            lhsT=kt_t, rhs=qt_sb[:, h, q0 + w:q0 + SG],
                            start=True, stop=True, skip_group_check=True,
                        )
                at2 = p2tmp.tile([P, 2, SG], bf, tag="at", bufs=8)
                nc.scalar.activation(at2, ps2t, Exp, scale=inv_sqrt_hd)
                ats.append(at2)
                dsum = p2tmp.tile([P, SG], f16, tag="dsum", bufs=4)
                nc.vector.tensor_add(dsum, at2[:, 0, :], at2[:, 1, :])
                dsums.append(dsum)
                # eager tree level 1: keeps the post-PV DVE tail short so
                # ctx-mul frees pctx before the next head's PV needs it
                if len(dsums) % 2 == 0:
                    d2 = p2tmp.tile([P, SG], f16, tag="dsum2", bufs=4)
                    nc.vector.tensor_add(d2, dsums[-2], dsums[-1])
                    dsums[-2:] = [d2]
                if j >= 2:
                    pump(1)

            # PV accumulation (exps have drained by now). Diagonal k-tiles
            # contribute nothing to q-columns left of their own block (the
            # masked at2 there is ~0), so trim the free range.
            for j in range(jmax):
                r = j - (g * SG // P)
                q_lo = 0 if r < 1 else r * P
                nc.tensor.matmul(
                    pctx[:, q_lo:SG],
                    lhsT=v_g[j // 4][:, j % 4, h * P:(h + 1) * P],
                    rhs=ats[j // 2][:, j % 2, q_lo:SG],
                    start=(j == 0),
                    stop=(j == jmax - 1),
                    skip_group_check=True,
                )
            # normalizer: full DVE tree down to one tile, then a single
            # ones-matmul (sum lands replicated on all partitions). Levels
            # alternate tag rings so a tree add never writes the slot one
            # of its own inputs occupies.
            lvl = dsums
            depth = 1  # eager level-1 already ran (outputs in dsum2 ring)
            while len(lvl) > 1:
                depth += 1
                tag = "dsum2" if depth % 2 else "dsum"
                nxt = []
                for i in range(0, len(lvl), 2):
                    if i + 1 < len(lvl):
                        d2 = p2tmp.tile([P, SG], f16, tag=tag, bufs=4)
                        nc.vector.tensor_add(d2, lvl[i], lvl[i + 1])
                        nxt.append(d2)
                    else:
                        nxt.append(lvl[i])
                lvl = nxt
            nc.tensor.matmul(
                psum_l, lhsT=ones_bf_sb[:], rhs=lvl[0], start=True, stop=True,
            )
            rec = recp.tile([P, SG], f32, tag="rec")
            nc.vector.reciprocal_approx_fast(rec, psum_l)
            nc.vector.tensor_mul(ctx_g[g][:, h, :], pctx, rec)
            pump(1)

        if g == 0:
            pump(len(filler))  # drain leftover V chains before po units queue
        # this g's out-proj becomes the next g's filler
        new_units = [po_unit(qt, dsl)
                     for qt in range(4 * g, 4 * (g + 1)) for dsl in range(nos)]
        if g == nsg - 1:
            for u in new_units:
                u()
        else:
            filler.extend(new_units)

    pump(len(filler))

    for pool in (ps_po, ps_l, ps_ctx, ps_pair, recp, outp, p2tmp, ctxp,
                 ph1, stores, const):
        pool.release()


def build_program(cfg=None):
    import concourse.bacc as bacc
    import concourse.mybir as mybir
    import concourse.tile as tile

    cfg = cfg or {"S": S, "D": D, "DOUT": DOUT}
    bf = mybir.dt.bfloat16
    nc = bacc.Bacc()
    io = {
        "xT": nc.dram_tensor("xT", [cfg["D"], cfg["S"]], bf, kind="ExternalInput"),
        "wq": nc.dram_tensor("wq", [cfg["D"], DG], bf, kind="ExternalInput"),
        "wk": nc.dram_tensor("wk", [cfg["D"], DG], bf, kind="ExternalInput"),
        "wv": nc.dram_tensor("wv", [cfg["D"], DG], bf, kind="ExternalInput"),
        "wo": nc.dram_tensor("wo", [DG, cfg["DOUT"]], bf, kind="ExternalInput"),
        "cosT": nc.dram_tensor("cosT", [P, cfg["S"]], bf, kind="ExternalInput"),
        "sinT": nc.dram_tensor("sinT", [P, cfg["S"]], bf, kind="ExternalInput"),
        "rot": nc.dram_tensor("rot", [P, P], bf, kind="ExternalInput"),
        "maskneg": nc.dram_tensor("maskneg", [P, HG, SG], bf, kind="ExternalInput"),
        "id": nc.dram_tensor("id", [P, P], bf, kind="ExternalInput"),
        "ones_bf": nc.dram_tensor("ones_bf", [P, P], bf, kind="ExternalInput"),
        "out": nc.dram_tensor(
            "out", [cfg["S"], cfg["DOUT"]], bf, kind="ExternalOutput"
        ),
    }
    with tile.TileContext(nc) as tc:
        _emit(tc, io, cfg)
    nc.finalize()
    return nc


def host_constants(s=S):
    inv = 1.0 / (10000.0 ** (np.arange(0, HEAD, 2, dtype=np.float32) / HEAD))
    pos = np.arange(s, dtype=np.float32)
    ang = pos[:, None] * inv[None, :]
    ang = np.concatenate([ang, ang], axis=-1)          # (s, HEAD)
    cosT = np.cos(ang).T.astype(BF16).copy()           # (HEAD, s)
    sinT = np.sin(ang).T.astype(BF16).copy()
    rot = np.zeros((HEAD, HEAD), np.float32)
    for dd in range(64):
        rot[dd, dd + 64] = -1.0
        rot[dd + 64, dd] = 1.0
    rotT = rot.T.astype(BF16).copy()                   # lhsT for out = rot @ q
    kk = np.arange(P)[:, None, None]
    rr = np.arange(HG)[None, :, None]
    qq = np.arange(SG)[None, None, :]
    maskneg = ((kk > qq - P * rr) * MASK_NEG).astype(BF16)  # (P, HG, SG)
    id128 = np.eye(P, dtype=np.float32).astype(BF16)
    ones_bf = np.ones((P, P), BF16)
    return cosT, sinT, rotT, maskneg, id128, ones_bf


def make_in_maps(x, W_query, W_key, W_value, W_out):
    cosT, sinT, rotT, maskneg, id128, ones_bf = host_constants()
    xTb = [np.ascontiguousarray(np.asarray(x[b]).T).astype(BF16) for b in range(B)]
    in_maps = []
    for core in range(N_CORES):
        b, g = divmod(core, HG)
        gsl = slice(g * DG, (g + 1) * DG)
        in_maps.append({
            "xT": xTb[b],
            "wq": np.asarray(W_query)[:, gsl].astype(BF16).copy(),
            "wk": np.asarray(W_key)[:, gsl].astype(BF16).copy(),
            "wv": np.asarray(W_value)[:, gsl].astype(BF16).copy(),
            "wo": np.asarray(W_out)[gsl, :].astype(BF16).copy(),
            "cosT": cosT, "sinT": sinT, "rot": rotT, "maskneg": maskneg,
            "id": id128, "ones_bf": ones_bf,
        })
    return in_maps


def kernel(x, W_query, W_key, W_value, W_out):
    from concourse.bass_utils import run_bass_kernel_spmd

    x = np.asarray(x)
    in_dtype = x.dtype
    nc = build_program()
    in_maps = make_in_maps(x, W_query, W_key, W_value, W_out)
    res = run_bass_kernel_spmd(nc, in_maps, core_ids=list(range(N_CORES)))
    out = np.zeros((B, S, DOUT), np.float32)
    for core in range(N_CORES):
        b = core // HG
        out[b] += np.asarray(res.results[core]["out"], np.float32)
    return out.astype(in_dtype, copy=False)


# revision 3
# speedup vs baseline: 1.0038x; 1.0017x over previous
"""Multi-head causal attention with RoPE on 8 Trainium2 NeuronCores — v2.

Sharding: 2 (batch) x 4 (head-groups of 4 heads), as v1. Differences vs v1,
all aimed at a stall-free PE stream (TRN2 HAM drops the PE clock to 1.2GHz
for ~3us after every stall):

  - o-major startup: QK projections for the first q-group run as 8 parallel
    PSUM accumulation chains fed by paired-o DMA, so the PE starts ~1.5us in
  - causal mask folded into the scores matmul as an additive -512 pattern
    (identity stationary), removing the DVE mask-multiply from the
    exp -> PV critical path
  - out-proj of group g-1 (and leftover V chains for g=0) interleaved into
    group g's exp-throttled scores stream as PE filler
  - bf16 output (host accumulates partial sums in f32)
  - batched DMA issues (sync sequencer pays ~617ns per dma_start)
"""

import math
import sys

import numpy as np

try:
    import concourse.bass as bass  # noqa: F401
except Exception:
    sys.path.insert(0, "/opt/trn_rl_repo")

import ml_dtypes

P = 128
B = 2
S = 2048
D = 2048
H = 16
HEAD = 128
N_CORES = 8
HG = 4            # head groups (tensor-parallel dimension)
HPG = H // HG     # heads per group = 4
DG = HPG * HEAD   # group width = 512
SG = 512          # q-group (free dim) size
DOUT = 2048

BF16 = ml_dtypes.bfloat16

MASK_NEG = -512.0   # pre-scale additive mask; exp(-512/sqrt(128)) ~ 3e-20


def _emit(tc, io, cfg, sfx=""):
    """Emit the per-core program. io: dict of dram APs. cfg: sizes."""
    import concourse.mybir as mybir

    nc = tc.nc
    bf = mybir.dt.bfloat16
    f16 = mybir.dt.float16
    f32 = mybir.dt.float32
    Exp = mybir.ActivationFunctionType.Exp

    s = cfg["S"]
    d = cfg["D"]
    dout = cfg["DOUT"]
    di_t = d // P          # d_in k-tiles (16)
    st = s // P            # seq 128-tiles (16)
    nsg = s // SG          # seq 512-groups (4)
    nos = dout // SG       # out column slices (4)
    inv_sqrt_hd = 1.0 / math.sqrt(HEAD)

    xT = io["xT"].rearrange("(o p) s -> p o s", p=P)
    wq = io["wq"].rearrange("(o p) n -> p o n", p=P)
    wk = io["wk"].rearrange("(o p) n -> p o n", p=P)
    wv = io["wv"].rearrange("(o p) n -> p o n", p=P)
    wo = io["wo"].rearrange("(o p) n -> p o n", p=P)

    const = tc.alloc_tile_pool(name="const" + sfx, bufs=1)
    stores = tc.alloc_tile_pool(name="stores" + sfx, bufs=1)

    # ---- constants ----
    cos_sb = const.tile([P, s], bf, tag="cos")
    sin_sb = const.tile([P, s], bf, tag="sin")
    rot_sb = const.tile([P, P], bf, tag="rot")
    mneg_sb = const.tile([P, HG, SG], bf, tag="mneg")
    id_sb = const.tile([P, P], bf, tag="id")
    ones_bf_sb = const.tile([P, P], bf, tag="ones_bf")
    wv_sb = const.tile([P, di_t, DG], bf, tag="wv")
    wo_sb = const.tile([P, HPG, dout], bf, tag="wo")

    # persistent activation stores. v is split per k-group so PV(g) reads
    # never false-share with later V-chain writes (subtile dep imprecision)
    qt_sb = stores.tile([P, HPG, s], bf, tag="qt")
    kt_sb = stores.tile([P, HPG, s], bf, tag="kt")
    v_g = [stores.tile([P, 4, DG], bf, tag=f"v{g}", name=f"v{g}")
           for g in range(nsg)]

    # ======================= phase 1 =======================
    ph1 = tc.alloc_tile_pool(name="ph1" + sfx, bufs=1)
    wqk = tc.alloc_tile_pool(name="wqk" + sfx, bufs=1)
    p1tmp = tc.alloc_tile_pool(name="p1tmp" + sfx, bufs=4)
    chp = tc.alloc_tile_pool(name="chp" + sfx, bufs=8, space="PSUM")

    # x^T split per s-group: chain reads of group g must not wait the DMA
    # of groups > g landing into the same tile
    xt_g = [ph1.tile([P, di_t, SG], bf, tag=f"xt{g}", name=f"xt{g}")
            for g in range(nsg)]
    wq_sb = wqk.tile([P, di_t, DG], bf, tag="wq")
    wk_sb = wqk.tile([P, di_t, DG], bf, tag="wk")

    # --- startup: o-major QK for g=0, all 4 heads, 8 psum chains;
    # paired-o DMA so the PE starts ~1.5us in and never waits again ---
    ch = {}
    for h in range(HPG):
        for qk in range(2):
            ch[(h, qk)] = chp.tile([P, SG], f32, tag="ch", name=f"ch{h}{qk}")
    # o-group sizes: singles first (fast first matmul), then wider groups
    # (3 DMA issues buy 4 o-steps of PE work, freeing the sync sequencer)
    ogroups = [(0,), (1,), (2, 3), (4, 5, 6, 7), (8, 9, 10, 11), (12, 13, 14, 15)]
    for og in ogroups:
        o0, o1 = og[0], og[-1] + 1
        nc.sync.dma_start(xt_g[0][:, o0:o1, :], xT[:, o0:o1, 0:SG])
        nc.sync.dma_start(wq_sb[:, o0:o1, :], wq[:, o0:o1, :])
        nc.sync.dma_start(wk_sb[:, o0:o1, :], wk[:, o0:o1, :])
        for oo in og:
            # q chains before k chains: the first four matmuls then need
            # only wq, hiding wk's issue+transfer
            for qk, w_sb in ((0, wq_sb), (1, wk_sb)):
                for h in range(HPG):
                    nc.tensor.matmul(
                        ch[(h, qk)],
                        lhsT=w_sb[:, oo, h * P:(h + 1) * P],
                        rhs=xt_g[0][:, oo, :],
                        start=(oo == 0),
                        stop=(oo == di_t - 1),
                    )

    # remaining DMA (big batched issues; transfers stream behind compute).
    # wv first: the V chains that cover the startup->steady qa drain need it
    # ~25us before anything else in this list.
    nc.sync.dma_start(wv_sb[:], wv[:])
    nc.sync.dma_start(cos_sb[:], io["cosT"][:])
    nc.sync.dma_start(sin_sb[:], io["sinT"][:])
    nc.sync.dma_start(rot_sb[:], io["rot"][:])
    nc.sync.dma_start(mneg_sb[:], io["maskneg"][:])
    nc.sync.dma_start(id_sb[:], io["id"][:])
    nc.sync.dma_start(ones_bf_sb[:], io["ones_bf"][:])
    nc.sync.dma_start(xt_g[1][:], xT[:, :, SG:2 * SG])
    nc.sync.dma_start(wo_sb[:], wo[:])
    for g in range(2, nsg):
        nc.sync.dma_start(xt_g[g][:], xT[:, :, g * SG:(g + 1) * SG])

    # --- RoPE for the startup chains + steady-state QK for g>=1 ---
    def emit_rope(qa, dst, hh, sl):
        pr = chp.tile([P, SG], f32, tag="ch")
        nc.tensor.matmul(pr, lhsT=rot_sb, rhs=qa, start=True, stop=True)
        t1 = p1tmp.tile([P, SG], bf, tag="t1")
        nc.vector.tensor_mul(t1, qa, cos_sb[:, sl])
        t2 = p1tmp.tile([P, SG], bf, tag="t2")
        nc.vector.tensor_mul(t2, pr, sin_sb[:, sl])
        nc.vector.tensor_add(dst[:, hh, sl], t1, t2)

    pending = []

    def queue_rope(pq, dst, hh, sl, window=4):
        qa = p1tmp.tile([P, SG], bf, tag="qa")
        nc.scalar.copy(qa, pq)
        pending.append((qa, dst, hh, sl))
        while len(pending) > window:
            emit_rope(*pending.pop(0))

    def emit_v(si, pool):
        pv = pool.tile([P, SG], f32, tag=pool is chp and "ch" or "po", name="pv")
        sg, sc = divmod(si, 4)
        for o in range(di_t):
            nc.tensor.matmul(
                pv,
                lhsT=xt_g[sg][:, o, sc * P:(sc + 1) * P],
                rhs=wv_sb[:, o, :],
                start=(o == 0),
                stop=(o == di_t - 1),
            )
        nc.scalar.copy(v_g[sg][:, sc, :], pv)

    sl0 = slice(0, SG)
    for h in range(HPG):
        queue_rope(ch[(h, 0)], qt_sb, h, sl0, window=99)
        queue_rope(ch[(h, 1)], kt_sb, h, sl0, window=99)
    # two V chains give the PE qa-independent work while the eight startup
    # qa copies drain serially on ACT (wv was the first big DMA issued)
    emit_v(0, chp)
    while len(pending) > 6:
        emit_rope(*pending.pop(0))
    emit_v(1, chp)
    while len(pending) > 4:
        emit_rope(*pending.pop(0))

    for g in range(1, nsg):
        sl = slice(g * SG, (g + 1) * SG)
        for h in range(HPG):
            for qk, (w_sb, dst) in enumerate(((wq_sb, qt_sb), (wk_sb, kt_sb))):
                pq = chp.tile([P, SG], f32, tag="ch")
                for o in range(di_t):
                    nc.tensor.matmul(
                        pq,
                        lhsT=w_sb[:, o, h * P:(h + 1) * P],
                        rhs=xt_g[g][:, o, :],
                        start=(o == 0),
                        stop=(o == di_t - 1),
                    )
                queue_rope(pq, dst, h, sl)
    while pending:
        emit_rope(*pending.pop(0))

    # --- V projection si 2..11 (0-1 ran at the transition; 12-15 = g0 filler)
    for si in range(2, 11):
        emit_v(si, chp)
    # last chain split column-wise: its psum->sbuf copies overlap, so the
    # phase-1 -> phase-2 psum pool boundary waits ~0.4us instead of ~1us
    for half in range(2):
        hsl = slice(half * (SG // 2), (half + 1) * (SG // 2))
        pvh = chp.tile([P, SG // 2], f32, tag="ch", name="pvh")
        for o in range(di_t):
            nc.tensor.matmul(
                pvh,
                lhsT=xt_g[2][:, o, 3 * P:4 * P],
                rhs=wv_sb[:, o, hsl],
                start=(o == 0),
                stop=(o == di_t - 1),
            )
        nc.scalar.copy(v_g[2][:, 3, hsl], pvh)

    chp.release()
    p1tmp.release()
    wqk.release()

    # ======================= phase 2 =======================
    ctxp = tc.alloc_tile_pool(name="ctx" + sfx, bufs=1)
    p2tmp = tc.alloc_tile_pool(name="p2tmp" + sfx, bufs=1)
    outp = tc.alloc_tile_pool(name="outp" + sfx, bufs=4)
    recp = tc.alloc_tile_pool(name="recp" + sfx, bufs=2)
    ps_pair = tc.alloc_tile_pool(name="ps_pair" + sfx, bufs=2, space="PSUM")
    ps_ctx = tc.alloc_tile_pool(name="ps_ctx" + sfx, bufs=1, space="PSUM")
    ps_po = tc.alloc_tile_pool(name="ps_po" + sfx, bufs=3, space="PSUM")

    # ctx split per q-group: out-proj filler for g-1 must not false-share
    # with the current group's ctx writes
    ctx_g = [ctxp.tile([P, HPG, SG], bf, tag=f"ctx{g}", name=f"ctx{g}")
             for g in range(nsg)]

    # ---- filler pump: units of PE work with no exp dependency ----
    filler = []
    reserve = [0]   # units held back for later groups

    def pump(n):
        for _ in range(min(n, len(filler) - reserve[0])):
            filler.pop(0)()

    nob = [0]

    def po_unit(qt, dsl):
        def emit():
            po = ps_po.tile([P, SG], f32, tag="po")
            qg, qc = divmod(qt, 4)
            for h in range(HPG):
                nc.tensor.matmul(
                    po,
                    lhsT=ctx_g[qg][:, h, qc * P:(qc + 1) * P],
                    rhs=wo_sb[:, h, dsl * SG:(dsl + 1) * SG],
                    start=(h == 0),
                    stop=(h == HPG - 1),
                )
            ob = outp.tile([P, SG], bf, tag="ob")
            # alternate the psum->sbuf copy between DVE and ACT
            nob[0] += 1
            if nob[0] % 2:
                nc.vector.tensor_copy(ob, po)
            else:
                nc.scalar.copy(ob, po)
            nc.sync.dma_start(
                io["out"][qt * P:(qt + 1) * P, dsl * SG:(dsl + 1) * SG], ob
            )
        return emit

    for si in range(12, 16):
        filler.append(lambda si=si: emit_v(si, ps_po))

    for g in range(nsg):
        qsl = slice(g * SG, (g + 1) * SG)
        jmax = min((g + 1) * SG // P, st)
        # hold back filler in g2 so the exp-bound g3 stream stays fed
        reserve[0] = 8 if g == 2 else 0
        for h in range(HPG):
            pctx = ps_ctx.tile([P, SG], f32, tag="pctx")

            # scores stream: paired MMs + one exp per [P, 2*SG]; filler
            # absorbs the exp cadence. Diagonal tiles split into three MMs
            # (mask starter over masked cols, score accum on the boundary
            # 128, fresh scores right of it) — constant 640 cyc/diag tile,
            # every psum address start=True'd exactly once.
            ats = []
            dsums = []
            for j in range(0, jmax, 2):
                ps2t = ps_pair.tile([P, 2, SG], f32, tag="pair")
                for jj in range(2):
                    r = (j + jj) - (g * SG // P)
                    kt_t = kt_sb[:, h, (j + jj) * P:(j + jj + 1) * P]
                    if r < 0:
                        nc.tensor.matmul(
                            ps2t[:, jj, :], lhsT=kt_t, rhs=qt_sb[:, h, qsl],
                            start=True, stop=True,
                        )
                        continue
                    w = P * (r + 1)
                    q0 = g * SG
                    nc.tensor.matmul(
                        ps2t[:, jj, 0:w],
                        lhsT=id_sb, rhs=mneg_sb[:, r, 0:w],
                        start=True, stop=False, skip_group_check=True,
                    )
                    nc.tensor.matmul(
                        ps2t[:, jj, P * r:w],
                        lhsT=kt_t, rhs=qt_sb[:, h, q0 + P * r:q0 + w],
                        start=False, stop=(w == SG), skip_group_check=True,
                    )
                    if w < SG:
                        nc.tensor.matmul(
                            ps2t[:, jj, w:SG],
                            lhsT=kt_t, rhs=qt_sb[:, h, q0 + w:q0 + SG],
                            start=True, stop=True, skip_group_check=True,
                        )
                at2 = p2tmp.tile([P, 2, SG], bf, tag="at", bufs=8)
                nc.scalar.activation(at2, ps2t, Exp, scale=inv_sqrt_hd)
                ats.append(at2)
                dsum = p2tmp.tile([P, SG], f16, tag="dsum", bufs=4)
                nc.vector.tensor_add(dsum, at2[:, 0, :], at2[:, 1, :])
                dsums.append(dsum)
                # eager tree level 1: keeps the post-PV DVE tail short so
                # ctx-mul frees pctx before the next head's PV needs it
                if len(dsums) % 2 == 0:
                    d2 = p2tmp.tile([P, SG], f16, tag="dsum2", bufs=4)
                    nc.vector.tensor_add(d2, dsums[-2], dsums[-1])
                    dsums[-2:] = [d2]
                if j >= 2:
                    pump(1)

            # PV accumulation (exps have drained by now). Diagonal k-tiles
            # contribute nothing to q-columns left of their own block (the
            # masked at2 there is ~0), so trim the free range.
            for j in range(jmax):
                r = j - (g * SG // P)
                q_lo = 0 if r < 1 else r * P
                nc.tensor.matmul(
                    pctx[:, q_lo:SG],
                    lhsT=v_g[j // 4][:, j % 4, h * P:(h + 1) * P],
                    rhs=ats[j // 2][:, j % 2, q_lo:SG],
                    start=(j == 0),
                    stop=(j == jmax - 1),
                    skip_group_check=True,
                )
            # normalizer: full DVE tree down to one tile, then a single
            # ones-matmul (sum lands replicated on all partitions). Levels
            # alternate tag rings so a tree add never writes the slot one
            # of its own inputs occupies.
            lvl = dsums
            depth = 1  # eager level-1 already ran (outputs in dsum2 ring)
            while len(lvl) > 1:
                depth += 1
                tag = "dsum2" if depth % 2 else "dsum"
                nxt = []
                for i in range(0, len(lvl), 2):
                    if i + 1 < len(lvl):
                        d2 = p2tmp.tile([P, SG], f16, tag=tag, bufs=4)
                        nc.vector.tensor_add(d2, lvl[i], lvl[i + 1])
                        nxt.append(d2)
                    else:
                        nxt.append(lvl[i])
                lvl = nxt
            # the normalizer sum borrows a pair-pool slot (first bank); by
            # now this head's last-but-one exp has freed it
            l_tile = ps_pair.tile([P, 2, SG], f32, tag="pair", name="l_tile")
            psum_l = l_tile[:, 0, :]
            nc.tensor.matmul(
                psum_l, lhsT=ones_bf_sb[:], rhs=lvl[0], start=True, stop=True,
            )
            rec = recp.tile([P, SG], f32, tag="rec")
            if g == nsg - 1 and h == HPG - 1:
                # last head: quarter-split so the tail out-proj (which reads
                # q-columns in order) starts after the first quarter
                for qq in range(4):
                    qs = slice(qq * P, (qq + 1) * P)
                    nc.vector.reciprocal_approx_fast(rec[:, qs], psum_l[:, qs])
                    nc.vector.tensor_mul(ctx_g[g][:, h, qs], pctx[:, qs], rec[:, qs])
            else:
                nc.vector.reciprocal_approx_fast(rec, psum_l)
                nc.vector.tensor_mul(ctx_g[g][:, h, :], pctx, rec)
            # g0's four V-filler units must last all four heads
            if not (g == 0 and h < 2):
                pump(1)

        if g == 0:
            pump(len(filler))  # drain leftover V chains before po units queue
        # this g's out-proj becomes the next g's filler
        new_units = [po_unit(qt, dsl)
                     for qt in range(4 * g, 4 * (g + 1)) for dsl in range(nos)]
        if g == nsg - 1:
            for u in new_units:
                u()
        else:
            filler.extend(new_units)

    pump(len(filler))

    for pool in (ps_po, ps_ctx, ps_pair, recp, outp, p2tmp, ctxp,
                 ph1, stores, const):
        pool.release()


def build_program(cfg=None):
    import concourse.bacc as bacc
    import concourse.mybir as mybir
    import concourse.tile as tile

    cfg = cfg or {"S": S, "D": D, "DOUT": DOUT}
    bf = mybir.dt.bfloat16
    nc = bacc.Bacc()
    io = {
        "xT": nc.dram_tensor("xT", [cfg["D"], cfg["S"]], bf, kind="ExternalInput"),
        "wq": nc.dram_tensor("wq", [cfg["D"], DG], bf, kind="ExternalInput"),
        "wk": nc.dram_tensor("wk", [cfg["D"], DG], bf, kind="ExternalInput"),
        "wv": nc.dram_tensor("wv", [cfg["D"], DG], bf, kind="ExternalInput"),
        "wo": nc.dram_tensor("wo", [DG, cfg["DOUT"]], bf, kind="ExternalInput"),
        "cosT": nc.dram_tensor("cosT", [P, cfg["S"]], bf, kind="ExternalInput"),
        "sinT": nc.dram_tensor("sinT", [P, cfg["S"]], bf, kind="ExternalInput"),
        "rot": nc.dram_tensor("rot", [P, P], bf, kind="ExternalInput"),
        "maskneg": nc.dram_tensor("maskneg", [P, HG, SG], bf, kind="ExternalInput"),
        "id": nc.dram_tensor("id", [P, P], bf, kind="ExternalInput"),
        "ones_bf": nc.dram_tensor("ones_bf", [P, P], bf, kind="ExternalInput"),
        "out": nc.dram_tensor(
            "out", [cfg["S"], cfg["DOUT"]], bf, kind="ExternalOutput"
        ),
    }
    with tile.TileContext(nc) as tc:
        _emit(tc, io, cfg)
    nc.finalize()
    return nc


def host_constants(s=S):
    inv = 1.0 / (10000.0 ** (np.arange(0, HEAD, 2, dtype=np.float32) / HEAD))
    pos = np.arange(s, dtype=np.float32)
    ang = pos[:, None] * inv[None, :]
    ang = np.concatenate([ang, ang], axis=-1)          # (s, HEAD)
    cosT = np.cos(ang).T.astype(BF16).copy()           # (HEAD, s)
    sinT = np.sin(ang).T.astype(BF16).copy()
    rot = np.zeros((HEAD, HEAD), np.float32)
    for dd in range(64):
        rot[dd, dd + 64] = -1.0
        rot[dd + 64, dd] = 1.0
    rotT = rot.T.astype(BF16).copy()                   # lhsT for out = rot @ q
    kk = np.arange(P)[:, None, None]
    rr = np.arange(HG)[None, :, None]
    qq = np.arange(SG)[None, None, :]
    maskneg = ((kk > qq - P * rr) * MASK_NEG).astype(BF16)  # (P, HG, SG)
    id128 = np.eye(P, dtype=np.float32).astype(BF16)
    ones_bf = np.ones((P, P), BF16)
    return cosT, sinT, rotT, maskneg, id128, ones_bf


def make_in_maps(x, W_query, W_key, W_value, W_out):
    cosT, sinT, rotT, maskneg, id128, ones_bf = host_constants()
    xTb = [np.ascontiguousarray(np.asarray(x[b]).T).astype(BF16) for b in range(B)]
    in_maps = []
    for core in range(N_CORES):
        b, g = divmod(core, HG)
        gsl = slice(g * DG, (g + 1) * DG)
        in_maps.append({
            "xT": xTb[b],
            "wq": np.asarray(W_query)[:, gsl].astype(BF16).copy(),
            "wk": np.asarray(W_key)[:, gsl].astype(BF16).copy(),
            "wv": np.asarray(W_value)[:, gsl].astype(BF16).copy(),
            "wo": np.asarray(W_out)[gsl, :].astype(BF16).copy(),
            "cosT": cosT, "sinT": sinT, "rot": rotT, "maskneg": maskneg,
            "id": id128, "ones_bf": ones_bf,
        })
    return in_maps


def kernel(x, W_query, W_key, W_value, W_out):
    from concourse.bass_utils import run_bass_kernel_spmd

    x = np.asarray(x)
    in_dtype = x.dtype
    nc = build_program()
    in_maps = make_in_maps(x, W_query, W_key, W_value, W_out)
    res = run_bass_kernel_spmd(nc, in_maps, core_ids=list(range(N_CORES)))
    out = np.zeros((B, S, DOUT), np.float32)
    for core in range(N_CORES):
        b = core // HG
        out[b] += np.asarray(res.results[core]["out"], np.float32)
    return out.astype(in_dtype, copy=False)
